# revision 3
# baseline (speedup 1.0000x reference)
"""BiMamba block on 8 Trainium2 NeuronCores via Bass/Tile — v2.

Sharding (SPMD, one shared NEFF, no collectives):
  core c: dir = c//4 (0=fwd, 1=bwd), batch = (c//2)%2, half = c%2.
Each core runs the full mamba pipeline for one (dir, batch) pair on its
half of d_inner (scan channels are independent), computing the full-d_inner
xi/conv/x_proj path locally (dt/B/C need the full d_inner contraction).
The d_inner axis is permuted per core so its own half is always blocks 0..7.
out_proj and the final output_proj are folded into one matrix on the host
(W_eff = out_w_half @ proj_w_dir, scaled by K to stay fp16-normal); each
core emits a partial (d_model, L) which the host sums / unreverses.

v2 structure (vs v1):
  - conv computed on DVE (4-tap tensor_scalar chain), not PE diagonals.
  - silu(z) read directly from PSUM by ACT (no zT buffer/evictions).
  - selective scan runs in two t-halves of 512 so the full 8-block y
    accumulator fits in PSUM (8 banks); scan state is carried across the
    halves by injecting dA0*carry into b at each block-start column.
  - the 16 scan chains (one per state n) are split between the DVE and
    Pool (gpsimd) engines, which both implement tensor_tensor_scan.
  - y is accumulated entirely in PSUM via identity matmuls; the D-skip
    seed comes from a diagonal-matrix matmul over xc; the z-gate is
    applied during PSUM eviction.
"""

import numpy as np

B, L, D = 2, 1024, 1024
DI, DH, NST, RNK = 2048, 1024, 16, 64
NBLK = DH // 128           # 8 d-blocks per half
NBLK_F = DI // 128         # 16 d-blocks full
HL = L // 2                # 512, t-half length
F16 = np.float16
K_OUT = 256.0              # fp16-normal scale for the folded out weight

# chains (state index n) whose b/hC mults run on the Pool engine; rest on DVE.
# Pool shares SBUF ports with DVE: concurrent Pool tensor_tensor slows DVE
# mults ~4.5x, so Pool offload is a net loss — keep empty.
POOL_CHAINS = frozenset()
XI_FP8 = False       # fp8e4 DoubleRow xi: 2x PE but rel err 3.9% > 2e-2 gate
W8_SCALE = 64.0      # fp8 weight scale (keeps w out of fp8-subnormal range)
SOFTPLUS = False     # no Softplus table on this HW build; use Exp+Ln

_CACHE = {}


def _build_module(a_imm=None, pool_chains=POOL_CHAINS, xi_fp8=XI_FP8,
                  softplus=SOFTPLUS):
    import concourse.bass as bass
    import concourse.mybir as mybir
    from concourse import bacc
    from concourse.tile import TileContext

    dt = mybir.dt
    AF = mybir.ActivationFunctionType
    OP = mybir.AluOpType

    nc = bacc.Bacc("TRN2", target_bir_lowering=False, debug=False)

    # ---- DRAM I/O ----
    xT_d = nc.dram_tensor("xT", (D, L), dt.float16, kind="ExternalInput")
    xi_dt = dt.float8e4 if xi_fp8 else dt.float16
    xT8_d = None
    if xi_fp8:
        xT8_d = nc.dram_tensor("xT8", (D, L), dt.float8e4, kind="ExternalInput")
    w_xi_d = nc.dram_tensor("w_xi", (NBLK_F, 128, 8, 128), xi_dt, kind="ExternalInput")
    w_z_d = nc.dram_tensor("w_z", (D, DH), dt.float16, kind="ExternalInput")
    conv_w_d = nc.dram_tensor("conv_w", (4, DI), dt.float32, kind="ExternalInput")
    conv_b_d = nc.dram_tensor("conv_b", (DI,), dt.float32, kind="ExternalInput")
    xp_w_d = nc.dram_tensor("xp_w", (DI, 128), dt.float16, kind="ExternalInput")
    dt_w_d = nc.dram_tensor("dt_w", (RNK, DH), dt.float16, kind="ExternalInput")
    dt_b_d = nc.dram_tensor("dt_b", (DH,), dt.float32, kind="ExternalInput")
    A_d = None
    if a_imm is None:
        A_d = nc.dram_tensor("A", (DH, NST), dt.float32, kind="ExternalInput")
    dskip_diag_d = nc.dram_tensor("dskip_diag", (NBLK * 128, 128), dt.float16, kind="ExternalInput")
    w_oe_d = nc.dram_tensor("w_oe", (DH, D), dt.float16, kind="ExternalInput")
    ident_d = nc.dram_tensor("ident", (128, 128), dt.float16, kind="ExternalInput")
    pT_d = nc.dram_tensor("pT", (D, L), dt.float32, kind="ExternalOutput")

    with TileContext(nc) as tc:
        const = tc.alloc_tile_pool(name="const", bufs=1)
        persist = tc.alloc_tile_pool(name="persist", bufs=1)
        dram = tc.alloc_tile_pool(name="dram", bufs=1, space="DRAM")
        bc_stage = dram.tile([2 * NST, L], dt.float16)

        # ---- constants / small tensors ----
        conv_w_sb = const.tile([128, 4, NBLK_F], dt.float32)
        nc.sync.dma_start(conv_w_sb, conv_w_d.ap().rearrange("j (g p) -> p j g", p=128))
        conv_b_sb = const.tile([128, NBLK_F], dt.float32)
        nc.sync.dma_start(conv_b_sb, conv_b_d.ap().rearrange("(g p) -> p g", p=128))
        xp_w_sb = const.tile([128, NBLK_F, 128], dt.float16)
        nc.sync.dma_start(xp_w_sb, xp_w_d.ap().rearrange("(g p) j -> p g j", p=128))
        dt_w_sb = const.tile([RNK, DH], dt.float16)
        nc.sync.dma_start(dt_w_sb, dt_w_d.ap())
        dt_b_sb = const.tile([128, NBLK], dt.float32)
        nc.sync.dma_start(dt_b_sb, dt_b_d.ap().rearrange("(g p) -> p g", p=128))
        A_sb = None
        if a_imm is None:
            A_sb = const.tile([128, NBLK, NST], dt.float32)
            nc.sync.dma_start(A_sb, A_d.ap().rearrange("(g p) n -> p g n", p=128))
        # dskip/ident/w_oe are needed only from phase B/C on; issue them on
        # the ACT DGE so they never delay phase-A weights on the SP queue.
        dskip_diag_sb = const.tile([128, NBLK, 128], dt.float16)
        nc.scalar.dma_start(
            dskip_diag_sb, dskip_diag_d.ap().rearrange("(g k) c -> k g c", k=128))
        w_oe_sb = const.tile([128, 8, D], dt.float16)
        w_oe_r = w_oe_d.ap().rearrange("(k p) m -> p k m", p=128)
        for k in range(8):
            nc.scalar.dma_start(w_oe_sb[:, k], w_oe_r[:, k])
        ident_sb = const.tile([128, 128], dt.float16)
        nc.scalar.dma_start(ident_sb, ident_d.ap())
        BT = const.tile([NST, L], dt.float16)
        CT = const.tile([NST, L], dt.float16)
        dtrT = const.tile([RNK, L], dt.float16)

        # ---- persistent activations ----
        dtT = persist.tile([128, NBLK, L], dt.float16)     # softplus dt
        u2 = persist.tile([128, NBLK * L], dt.float16)     # dt * xc (own half)
        u3 = u2.rearrange("p (g t) -> p g t", g=NBLK)
        sz = persist.tile([128, NBLK, L], dt.float16)      # silu(z)
        y2 = persist.tile([128, NBLK, L], dt.float16)      # gated scan output
        xc_own = persist.tile([128, NBLK, L], dt.float16)  # conv out, own half
        carry = persist.tile([128, NST, NBLK], dt.float16) # scan state at t=511

        # ================= phase A: in_proj, conv, x_proj, z, dt ==============
        psumA = tc.alloc_tile_pool(name="psumA", bufs=6, space="PSUM")
        pha = tc.alloc_tile_pool(name="pha", bufs=1)
        xT_sb = pha.tile([128, 8, L], dt.float16)
        xT_r = xT_d.ap().rearrange("(k p) t -> p k t", p=128)
        for k in range(8):
            nc.sync.dma_start(xT_sb[:, k], xT_r[:, k])
        xT8_sb = None
        if xi_fp8:
            xT8_sb = pha.tile([128, 8, L], dt.float8e4)
            xT8_r = xT8_d.ap().rearrange("(k p) t -> p k t", p=128)
            for k in range(8):
                nc.sync.dma_start(xT8_sb[:, k], xT8_r[:, k])
        w_z_sb = pha.tile([128, 8, DH], dt.float16)
        w_z_r = w_z_d.ap().rearrange("(k p) m -> p k m", p=128)
        for k in range(8):
            nc.sync.dma_start(w_z_sb[:, k], w_z_r[:, k])
        xc_oth = pha.tile([128, NBLK, L], dt.float16)

        # xi blocks stream through the PE (fp8 DoubleRow: 2 k-blocks per
        # matmul at 0.5 cyc/row, weights pre-scaled by W8_SCALE to stay out
        # of the fp8 subnormal range, undone in the PSUM-eviction copy);
        # conv runs on DVE as a 4-tap per-partition-scalar chain.
        for m in range(NBLK_F):
            wxi_m = pha.tile([128, 8, 128], xi_dt, tag="wxi", bufs=3)
            nc.sync.dma_start(wxi_m, w_xi_d.ap()[m])
            xi_pad = pha.tile([128, 1027], dt.float16, tag="xi_pad", bufs=3)
            nc.vector.memset(xi_pad[:, 0:3], 0.0)
            for h in range(2):
                ps = psumA.tile([128, 512], dt.float32, tag="mm")
                if xi_fp8:
                    for j in range(4):
                        nc.tensor.matmul(
                            ps,
                            wxi_m[:, 2 * j:2 * j + 2, :],
                            xT8_sb[:, 2 * j:2 * j + 2, h * 512:(h + 1) * 512],
                            start=(j == 0),
                            stop=(j == 3),
                            perf_mode=mybir.MatmulPerfMode.DoubleRow,
                        )
                else:
                    for k in range(8):
                        nc.tensor.matmul(
                            ps,
                            wxi_m[:, k, :],
                            xT_sb[:, k, h * 512:(h + 1) * 512],
                            start=(k == 0),
                            stop=(k == 7),
                        )
                nc.scalar.activation(
                    xi_pad[:, 3 + h * 512: 3 + (h + 1) * 512], ps, AF.Copy,
                    scale=(1.0 / W8_SCALE) if xi_fp8 else 1.0)
            acc = pha.tile([128, L], dt.float16, tag="conv_acc", bufs=3)
            nc.vector.tensor_scalar(
                acc, xi_pad[:, 0:L], conv_w_sb[:, 0, m:m + 1], None, OP.mult)
            for j in range(1, 4):
                nc.vector.scalar_tensor_tensor(
                    acc, xi_pad[:, j:j + L], conv_w_sb[:, j, m:m + 1], acc,
                    OP.mult, OP.add)
            xc_dst = xc_own if m < NBLK else xc_oth
            nc.scalar.activation(
                xc_dst[:, m % NBLK, :], acc, AF.Silu, bias=conv_b_sb[:, m:m + 1])

        # z^T = w_z^T @ x^T ; silu applied straight out of PSUM by ACT
        for m in range(NBLK):
            for h in range(2):
                ps = psumA.tile([128, 512], dt.float32, tag="mm")
                for k in range(8):
                    nc.tensor.matmul(
                        ps,
                        w_z_sb[:, k, m * 128:(m + 1) * 128],
                        xT_sb[:, k, h * 512:(h + 1) * 512],
                        start=(k == 0),
                        stop=(k == 7),
                    )
                nc.scalar.activation(sz[:, m, h * 512:(h + 1) * 512], ps, AF.Silu)

        # dbc^T = xp_w^T @ xc^T -> [96, L] (dt_raw / B / C rows)
        for h in range(2):
            ps96 = psumA.tile([128, 512], dt.float32, tag="mm")
            for k in range(NBLK_F):
                xc_src = xc_own if k < NBLK else xc_oth
                nc.tensor.matmul(
                    ps96,
                    xp_w_sb[:, k, :],
                    xc_src[:, k % NBLK, h * 512:(h + 1) * 512],
                    start=(k == 0),
                    stop=(k == NBLK_F - 1),
                )
            nc.vector.tensor_copy(dtrT[:, h * 512:(h + 1) * 512], ps96[0:RNK, :])
            nc.vector.tensor_copy(BT[:, h * 512:(h + 1) * 512], ps96[RNK:RNK + NST, :])
            nc.vector.tensor_copy(CT[:, h * 512:(h + 1) * 512], ps96[96:96 + NST, :])
        nc.sync.dma_start(bc_stage[0:NST, :], BT)
        nc.sync.dma_start(bc_stage[NST:2 * NST, :], CT)

        # dt^T = softplus(dt_w^T @ dt_raw^T + dt_b); Softplus table if the HW
        # has it, else Ln(Exp(v)+1) with all Exps grouped before all Lns.
        ev = None if softplus else pha.tile([128, NBLK, L], dt.float16)
        for m in range(NBLK):
            for h in range(2):
                ps = psumA.tile([128, 512], dt.float32, tag="mm")
                nc.tensor.matmul(
                    ps,
                    dt_w_sb[:, m * 128:(m + 1) * 128],
                    dtrT[:, h * 512:(h + 1) * 512],
                    start=True,
                    stop=True,
                )
                if softplus:
                    nc.scalar.activation(
                        dtT[:, m, h * 512:(h + 1) * 512], ps, AF.Softplus,
                        bias=dt_b_sb[:, m:m + 1])
                else:
                    nc.scalar.activation(
                        ev[:, m, h * 512:(h + 1) * 512], ps, AF.Exp,
                        bias=dt_b_sb[:, m:m + 1])
        if not softplus:
            for m in range(NBLK):
                nc.scalar.activation(dtT[:, m, :], ev[:, m, :], AF.Ln, bias=1.0)

        # u = dt * xc (own half only)
        nc.vector.tensor_tensor(u3, dtT, xc_own, OP.mult)

        psumA.release()
        pha.release()

        # ====== phase B: selective scan over n, two d-block groups of 4 ======
        # Each group scans its 4 blocks over the FULL sequence (so there is
        # no cross-scan state carry at all); the 4-block fp32 y accumulator
        # fills all 8 PSUM banks for the duration of the group.
        psumY = tc.alloc_tile_pool(name="psumY", bufs=1, space="PSUM")
        phb = tc.alloc_tile_pool(name="phb", bufs=1)
        GB = 4                                      # blocks per group
        dtT2 = dtT.rearrange("p g t -> p (g t)")
        u2v = u2
        sz2 = sz.rearrange("p g t -> p (g t)")
        y22 = y2.rearrange("p g t -> p (g t)")

        for bg in range(2):
            g0 = bg * GB
            y_ps = psumY.tile([128, GB, L], dt.float32, tag="y", bufs=1)
            # D-skip seed: y = diag(D) @ xc for each block
            for g in range(GB):
                for hf in range(2):
                    nc.tensor.matmul(
                        y_ps[:, g, hf * 512:(hf + 1) * 512],
                        dskip_diag_sb[:, g0 + g, :],
                        xc_own[:, g0 + g, hf * 512:(hf + 1) * 512],
                        start=True, stop=False, skip_group_check=True,
                    )

            # DVE stream is software-pipelined one chain deep:
            #   ..., scan_n, b_{n+1}, hC_n, ...
            # so the scan's SBUF write-drain overlaps the independent b-mult
            # instead of stalling the immediately-dependent h*C multiply.
            def issue_b(n):
                b_rep = phb.tile([128, L], dt.float16, tag="brep", bufs=3)
                nc.sync.dma_start(
                    b_rep, bc_stage[n:n + 1, :].broadcast_to((128, L)))
                b = phb.tile([128, GB * L], dt.float16, tag="b", bufs=3)
                b3 = b.rearrange("p (g t) -> p g t", g=GB)
                nc.vector.tensor_tensor(
                    b3, u3[:, g0:g0 + GB, :],
                    b_rep.unsqueeze(1).broadcast_to((128, GB, L)), OP.mult)
                return b

            b_next = issue_b(0)
            for n in range(NST):
                c_rep = phb.tile([128, L], dt.float16, tag="crep", bufs=3)
                nc.sync.dma_start(
                    c_rep, bc_stage[NST + n:NST + n + 1, :].broadcast_to((128, L)))

                # dA = exp(A_n * dt); block-start columns zeroed (chain reset)
                dA = phb.tile([128, GB * L], dt.float16, tag="dA", bufs=3)
                if a_imm is not None:
                    nc.scalar.activation(
                        dA, dtT2[:, g0 * L:(g0 + GB) * L], AF.Exp,
                        scale=float(a_imm[n]))
                else:
                    dA3 = dA.rearrange("p (g t) -> p g t", g=GB)
                    for g in range(GB):
                        nc.scalar.activation(
                            dA3[:, g, :], dtT[:, g0 + g, :], AF.Exp,
                            scale=A_sb[:, g0 + g, n:n + 1])
                nc.vector.memset(dA[:, 0:GB * L:L], 0.0)

                b = b_next
                h = phb.tile([128, GB * L], dt.float16, tag="h", bufs=3)
                nc.vector.tensor_tensor_scan(h, dA, b, 0.0, OP.mult, OP.add)
                if n + 1 < NST:
                    b_next = issue_b(n + 1)

                h3 = h.rearrange("p (g t) -> p g t", g=GB)
                nc.vector.tensor_tensor(
                    h3, h3, c_rep.unsqueeze(1).broadcast_to((128, GB, L)),
                    OP.mult)
                for g in range(GB):
                    for hf in range(2):
                        nc.tensor.matmul(
                            y_ps[:, g, hf * 512:(hf + 1) * 512], ident_sb,
                            h[:, g * L + hf * 512: g * L + (hf + 1) * 512],
                            start=False, stop=(n == NST - 1),
                            skip_group_check=True,
                        )

            # evict + z-gate: y2 = y_ps * silu(z)
            nc.vector.tensor_tensor(
                y22[:, g0 * L:(g0 + GB) * L],
                y_ps.rearrange("p g t -> p (g t)"),
                sz2[:, g0 * L:(g0 + GB) * L], OP.mult)

        phb.release()
        psumY.release()

        # ================= phase C: folded out_proj ==========================
        psumC = tc.alloc_tile_pool(name="psumC", bufs=6, space="PSUM")
        phc = tc.alloc_tile_pool(name="phc", bufs=1)
        pT_r = pT_d.ap().rearrange("(k p) t -> p k t", p=128)
        for m in range(8):
            pT_m = phc.tile([128, L], dt.float32, tag="pT", bufs=3)
            for h in range(2):
                ps = psumC.tile([128, 512], dt.float32, tag="mm")
                for k in range(8):
                    nc.tensor.matmul(
                        ps,
                        w_oe_sb[:, k, m * 128:(m + 1) * 128],
                        y2[:, k, h * 512:(h + 1) * 512],
                        start=(k == 0),
                        stop=(k == 7),
                    )
                nc.scalar.copy(pT_m[:, h * 512:(h + 1) * 512], ps)
            nc.sync.dma_start(pT_r[:, m], pT_m)
        phc.release()
        psumC.release()
        dram.release()
        persist.release()
        const.release()

    nc.compile()
    return nc


def _wxi_layout(w_xi):
    """(D, DI) -> (16, 128, 8, 128): [m, p, k, c] = w[k*128+p, m*128+c]."""
    return np.ascontiguousarray(
        w_xi.reshape(8, 128, NBLK_F, 128).transpose(2, 1, 0, 3), dtype=F16)


def _wxi_layout_fp8(w_xi):
    """DoubleRow fp8 layout: [m, p, (j i), c] = w[(2j+i)*128+p, m*128+c],
    scaled by W8_SCALE (undone after the PSUM accumulate)."""
    import ml_dtypes
    w = (w_xi * W8_SCALE).reshape(8, 128, NBLK_F, 128).transpose(2, 1, 0, 3)
    return np.ascontiguousarray(w).astype(ml_dtypes.float8_e4m3)


def _dskip_diag(dskip):
    """(DH,) -> (8*128, 128) per-block diagonal matrices for the PE seed."""
    out = np.zeros((NBLK, 128, 128), F16)
    idx = np.arange(128)
    for g in range(NBLK):
        out[g, idx, idx] = dskip[g * 128:(g + 1) * 128].astype(F16)
    return out.reshape(NBLK * 128, 128)


def _a_imm(inputs):
    al = np.float64(inputs["A_log"])
    A = (-np.exp(al)).astype(np.float32)
    row = A[0, 0]
    if np.array_equal(A, np.broadcast_to(row, A.shape)):
        return tuple(float(v) for v in row)
    return None


def _pad_xp(xp):
    """(DI, 96) -> (DI, 128) with C cols moved to 96 (PSUM partition-start
    alignment)."""
    out = np.zeros((DI, 128), F16)
    out[:, :RNK + NST] = xp[:, :RNK + NST]
    out[:, 96:96 + NST] = xp[:, RNK + NST:]
    return out


def _prep_core_inputs(inputs, c, with_A):
    dr, b, half = c // 4, (c // 2) % 2, c % 2
    s0 = half * DH
    perm = np.r_[DH:DI, 0:DH] if half == 1 else np.r_[0:DI]

    x = inputs["x"][b]
    if dr == 1:
        x = x[::-1]
    in_w = inputs["in_w"][dr]
    w_out = inputs["out_w"][dr][s0:s0 + DH].astype(np.float32)
    w_proj = inputs["proj_w"][dr * D:(dr + 1) * D].astype(np.float32)
    w_oe = (w_out @ w_proj) * K_OUT

    m = {
        "xT": np.ascontiguousarray(x.T, dtype=F16),
        "w_xi": (_wxi_layout_fp8(in_w[:, :DI][:, perm]) if XI_FP8
                 else _wxi_layout(in_w[:, :DI][:, perm])),
        "w_z": np.ascontiguousarray(in_w[:, DI + s0:DI + s0 + DH], dtype=F16),
        "conv_w": np.ascontiguousarray(inputs["conv_w"][dr][perm].T, dtype=np.float32),
        "conv_b": np.ascontiguousarray(inputs["conv_b"][dr][perm], dtype=np.float32),
        "xp_w": _pad_xp(inputs["xp_w"][dr][perm]),
        "dt_w": np.ascontiguousarray(inputs["dt_w"][dr][:, s0:s0 + DH], dtype=F16),
        "dt_b": np.ascontiguousarray(inputs["dt_b"][dr][s0:s0 + DH], dtype=np.float32),
        "dskip_diag": _dskip_diag(inputs["D"][dr][s0:s0 + DH]),
        "w_oe": np.ascontiguousarray(w_oe, dtype=F16),
        "ident": np.eye(128, dtype=F16),
    }
    if XI_FP8:
        import ml_dtypes
        m["xT8"] = np.ascontiguousarray(x.T).astype(ml_dtypes.float8_e4m3)
    if with_A:
        A_full = -np.exp(np.float64(inputs["A_log"][dr])).astype(np.float32)
        m["A"] = np.ascontiguousarray(A_full[s0:s0 + DH], dtype=np.float32)
    return m


def _gather(inputs, results):
    out = np.zeros((B, L, D), np.float32)
    for c, res in enumerate(results):
        dr, b = c // 4, (c // 2) % 2
        p = res["pT"].T * (1.0 / K_OUT)
        if dr == 1:
            p = p[::-1]
        out[b] += p
    out += inputs["proj_b"]
    return out


def kernel(**inputs):
    inputs = {k: np.asarray(v) for k, v in inputs.items()}
    a_imm = _a_imm(inputs)
    key = ("nc", a_imm)
    if key not in _CACHE:
        _CACHE[key] = _build_module(a_imm=a_imm)
    nc = _CACHE[key]
    in_maps = [_prep_core_inputs(inputs, c, with_A=a_imm is None) for c in range(8)]
    from concourse.bass_utils import run_bass_kernel_spmd
    res = run_bass_kernel_spmd(nc, in_maps, core_ids=list(range(8)))
    return _gather(inputs, res.results)
